# revision 20
# baseline (speedup 1.0000x reference)
"""Trainium2 Bass kernel for nn_Cross_attention_3 (sparse_attention).

Sharding: the (D, H*W) plane is unfolded into 9x9 patches; D=72 gives 8
patch-row blocks of 9 rows — exactly one per NeuronCore.  The only
cross-core dependency is the AdaptiveAvgPool over the patch axis
(bins of 128 patches straddle core boundaries); cores exchange 8-patch
half-block partial sums via a 3.4MB AllGather.

The two MLP linears have no nonlinearity between them, so they collapse
into a single 81x81 matrix; the conv bias rides along as an 82nd
contraction row whose rhs holds b[c].  The 1x1x1 conv is computed with
the patch data as the matmul's stationary operand, so its output lands
directly in (patch-element, channel) layout — the transpose the rest of
the pipeline needs comes for free.  Patches are packed in slot-halves
(slot 0 = patches 0..647, slot 1 = 648..1295) so pooling windows and
attention output runs stay contiguous.
"""

import os
import sys

import numpy as np

try:
    import ml_dtypes
except ImportError:
    ml_dtypes = None

try:
    import concourse.bacc as _  # noqa: F401
except ImportError:  # container default path
    sys.path.insert(0, "/opt/trn_rl_repo")

import concourse.bacc as bacc
import concourse.mybir as mybir
from concourse.bass_utils import run_bass_kernel_spmd
from concourse.tile import TileContext

P = 9
P2 = 81
C = 64
D = 72
H = W = 108
HW = H * W
NCORES = 8
LLOC = HW // P   # 1296 patches per core
LP = LLOC // 2   # 648 patch pairs per core (slot halves)

NLP = 24                     # chunk size in pairs, both passes
NCH_A = LP // NLP            # 27 chunks
RING_B = 216                 # pass-B ring, in pairs
NRING = LP // RING_B         # 3
NSUB = RING_B // NLP         # 9
NHALF = LP // 8              # 81 half-blocks (8 patches) per slot

F32 = mybir.dt.float32
BF16 = mybir.dt.bfloat16

_cache = {}


def _build_nc():
    nc = bacc.Bacc(None, target_bir_lowering=False, debug=False)
    xp_d = nc.declare_dram_parameter("xp", [128, LP, P2], BF16, isOutput=False)
    yp_d = nc.declare_dram_parameter("yp", [128, LP, P2], BF16, isOutput=False)
    wi_d = nc.declare_dram_parameter("wi", [128, 128], BF16, isOutput=False)
    wf_d = nc.declare_dram_parameter("wf", [128, 128], BF16, isOutput=False)
    wm_d = nc.declare_dram_parameter("wm", [82, P2], BF16, isOutput=False)
    bi_d = nc.declare_dram_parameter("bi", [1, NLP * 128], BF16, isOutput=False)
    bf_d = nc.declare_dram_parameter("bf", [1, NLP * 128], BF16, isOutput=False)
    out_d = nc.declare_dram_parameter("out", [P2, C, 2 * LP], BF16, isOutput=True)

    # per-slot 8-patch half-block sums; gathered across cores
    gath_d = nc.dram_tensor("gath", [NCORES, P2, C, NHALF], F32,
                            addr_space="Shared")

    with nc.allow_low_precision("bf16 compute pipeline"), TileContext(nc) as tc:
        with (
            tc.tile_pool(name="const", bufs=1) as constp,
            tc.tile_pool(name="stage", bufs=4) as stagep,
            tc.tile_pool(name="mlps", bufs=2) as mlpp,
            tc.tile_pool(name="psconv", bufs=4, space="PSUM") as psconv,
            tc.tile_pool(name="psmlp", bufs=2, space="PSUM") as psmlp,
            tc.tile_pool(name="dram", bufs=1, space="DRAM") as dramp,
        ):
            wi_sb = constp.tile([128, 128], BF16, tag="wi")
            wf_sb = constp.tile([128, 128], BF16, tag="wf")
            wm_sb = constp.tile([82, P2], BF16, tag="wm")
            pooled = constp.tile([P2, C, P2], BF16, tag="pooled")
            h_dram = dramp.tile([P2, C, NHALF], F32)
            nc.sync.dma_start(out=wi_sb[:, :], in_=wi_d[:, :])
            nc.sync.dma_start(out=wf_sb[:, :], in_=wf_d[:, :])
            nc.sync.dma_start(out=wm_sb[:, :], in_=wm_d[:, :])

            def conv_mlp(src_d, w_sb, b_d, lp0, dst, dst_lp0, dst_nlp, act_ix):
                """conv+MLP+lrelu for NLP pairs starting at pair lp0 of src_d.
                dst: (81, dst_nlp, 128) bf16 tile, (lp, sc)-major, written at
                lp offset dst_lp0."""
                st = stagep.tile([128, NLP, P2], BF16, tag="stage")
                nc.sync.dma_start(out=st[:, :, :], in_=src_d[:, lp0:lp0 + NLP, :])
                ms = mlpp.tile([82, NLP, 128], BF16, tag="ms")
                nc.sync.dma_start(
                    out=ms[81:82, :, :].rearrange("p a b -> p (a b)"),
                    in_=b_d[:, :],
                )
                # conv: 4 pairs per PSUM bank, N=128 each; one contiguous
                # evict per bank into ms (lp-major); DVE 2/3, ACT 1/3
                for pb in range(NLP // 4):
                    ps = psconv.tile([P2, 512], F32, tag="psc")
                    for j in range(4):
                        nc.tensor.matmul(
                            ps[0:P2, 128 * j:128 * (j + 1)],
                            st[:, 4 * pb + j, :], w_sb[:, :],
                            start=True, stop=True,
                        )
                    d = ms[0:P2, 4 * pb:4 * pb + 4, :].rearrange("p a b -> p (a b)")
                    if pb % 3 == 2:
                        nc.scalar.copy(d, ps[0:P2, 0:512])
                    else:
                        nc.vector.tensor_copy(d, ps[0:P2, 0:512])
                # MLP (+bias row) and LeakyReLU on ACT; 512 cols = 4 lp
                flat = ms[:, :, :].rearrange("p a b -> p (a b)")
                for q in range(NLP // 4):
                    mp = psmlp.tile([P2, 512], F32, tag="psm")
                    nc.tensor.matmul(
                        mp[0:P2, :], wm_sb[:, :], flat[:, 512 * q:512 * (q + 1)],
                        start=True, stop=True,
                    )
                    dq = dst[0:P2, dst_lp0 + 4 * q:dst_lp0 + 4 * q + 4,
                             :].rearrange("p a b -> p (a b)")
                    nc.scalar.activation(
                        dq, mp[0:P2, :],
                        mybir.ActivationFunctionType.Prelu, alpha=0.2,
                    )

            # ---------------- pass A: fea (y) + pooled ----------------------
            with (
                tc.tile_pool(name="feaout", bufs=2) as feap,
                tc.tile_pool(name="pooltmp", bufs=2) as ptp,
                tc.tile_pool(name="hsb", bufs=1) as hsbp,
                tc.tile_pool(name="comb", bufs=2) as combp,
            ):
                # h_sb: 8-patch half-block sums, (81, 162, 64): dim1 =
                # slot*81 + h, dim2 = c
                h_sb = hsbp.tile([P2, 2 * NHALF, C], F32, tag="hsb")
                s_sb = hsbp.tile([P2, C, NHALF], F32, tag="ssb")
                for ch in range(NCH_A):
                    fea = feap.tile([P2, NLP, 128], BF16, tag="fea")
                    conv_mlp(yp_d, wf_sb, bf_d, ch * NLP, fea, 0, NLP, ch)
                    # pairwise tree over lp: 24 -> 12 -> 6 -> 3 half-sums
                    t1 = ptp.tile([P2, 12, 128], BF16, tag="t1")
                    f2 = fea[0:P2, :, :].rearrange("p (a two) b -> p a two b", two=2)
                    nc.vector.tensor_tensor(
                        t1[0:P2, :, :], f2[:, :, 0, :], f2[:, :, 1, :],
                        mybir.AluOpType.add,
                    )
                    t2 = ptp.tile([P2, 6, 128], BF16, tag="t2")
                    t1s = t1[0:P2, :, :].rearrange("p (a two) b -> p a two b", two=2)
                    nc.vector.tensor_tensor(
                        t2[0:P2, :, :], t1s[:, :, 0, :], t1s[:, :, 1, :],
                        mybir.AluOpType.add,
                    )
                    t3 = ptp.tile([P2, 3, 128], F32, tag="t3")
                    t2s = t2[0:P2, :, :].rearrange("p (a two) b -> p a two b", two=2)
                    nc.vector.tensor_tensor(
                        t3[0:P2, :, :], t2s[:, :, 0, :], t2s[:, :, 1, :],
                        mybir.AluOpType.add,
                    )
                    # scatter the 3 half-sums per slot into h_sb
                    for slot in range(2):
                        nc.vector.tensor_copy(
                            h_sb[0:P2, NHALF * slot + 3 * ch:
                                 NHALF * slot + 3 * ch + 3, :],
                            t3[0:P2, :, 64 * slot:64 * slot + 64],
                        )
                # merge halves into 81 16-patch blocks (S), c-major for the
                # gather: S[e, c, s]
                hv = h_sb[0:P2, :, :]
                # s in [0, 40): slot0 pairs (2s, 2s+1)
                e0 = hv[:, 0:80, :].rearrange("p (a two) c -> p a two c", two=2)
                nc.vector.tensor_tensor(
                    s_sb[0:P2, :, 0:40].rearrange("p c s -> p s c"),
                    e0[:, :, 0, :], e0[:, :, 1, :], mybir.AluOpType.add,
                )
                # s = 40: slot0 h=80 + slot1 h=0
                nc.vector.tensor_tensor(
                    s_sb[0:P2, :, 40:41].rearrange("p c s -> p s c"),
                    hv[:, 80:81, :], hv[:, 81:82, :], mybir.AluOpType.add,
                )
                # s in [41, 81): slot1 pairs (1+2t, 2+2t)
                e1 = hv[:, 82:162, :].rearrange("p (a two) c -> p a two c", two=2)
                nc.vector.tensor_tensor(
                    s_sb[0:P2, :, 41:81].rearrange("p c s -> p s c"),
                    e1[:, :, 0, :], e1[:, :, 1, :], mybir.AluOpType.add,
                )
                nc.gpsimd.dma_start(out=h_dram[:, :, :], in_=s_sb[:, :, :])
                nc.gpsimd.collective_compute(
                    "AllGather",
                    mybir.AluOpType.bypass,
                    replica_groups=[list(range(NCORES))],
                    ins=[h_dram[:, :, :]],
                    outs=[gath_d[:, :, :, :]],
                )
                # bins of 128 = 8 consecutive global 16-blocks (s_g = 81k + s)
                for cc in range(8):
                    tcb = combp.tile([P2, 8, NCORES * NHALF], F32, tag="tcb")
                    for k in range(NCORES):
                        nc.gpsimd.dma_start(
                            out=tcb[0:P2, :, NHALF * k:NHALF * (k + 1)],
                            in_=gath_d[k, :, 8 * cc:8 * (cc + 1), :],
                        )
                    pr = combp.tile([P2, 8, P2], F32, tag="pr")
                    nc.vector.tensor_reduce(
                        pr[0:P2, :, :],
                        tcb[0:P2, :, :].rearrange("p c (j m) -> p c j m", m=8),
                        mybir.AxisListType.X,
                        mybir.AluOpType.add,
                    )
                    nc.vector.tensor_scalar_mul(
                        pooled[0:P2, 8 * cc:8 * (cc + 1), :], pr[0:P2, :, :],
                        1.0 / 128.0,
                    )

            # ---------------- pass B: img (x) + attention -------------------
            with (
                tc.tile_pool(name="imgring", bufs=2) as imgp,
                tc.tile_pool(name="attev", bufs=4) as attevp,
                tc.tile_pool(name="psatt", bufs=2, space="PSUM") as psatt,
            ):
                for ring in range(NRING):
                    img = imgp.tile([P2, RING_B, 128], BF16, tag="img")
                    for sub in range(NSUB):
                        conv_mlp(
                            xp_d, wi_sb, bi_d,
                            ring * RING_B + sub * NLP, img, sub * NLP, RING_B,
                            sub,
                        )
                    lp0 = ring * RING_B
                    ncols = RING_B * 2
                    for c in range(C):
                        ap = psatt.tile([P2, 512], F32, tag="psa")
                        # rhs cols (slot, lp): l = 648*slot + lp0 + lp
                        rhs = img[0:P2, :, :].rearrange(
                            "p l (s c) -> p c s l", s=2
                        )[:, c:c + 1, :, :]
                        nc.tensor.matmul(
                            ap[0:P2, 0:ncols], pooled[:, c:c + 1, :], rhs,
                            start=True, stop=True,
                        )
                        ev = attevp.tile([P2, 2, RING_B], BF16, tag="attev")
                        src = ap[0:P2, 0:ncols].rearrange("p (s l) -> p s l", s=2)
                        if c % 4 == 3:
                            nc.scalar.copy(ev[0:P2, :, :], src)
                        else:
                            nc.vector.tensor_copy(ev[0:P2, :, :], src)
                        # out[e, c, 648*slot + lp0 : +RING_B]
                        dstap = out_d[0:P2, c:c + 1, :].rearrange(
                            "p o (s l) -> p o s l", s=2
                        )[:, :, :, lp0:lp0 + RING_B]
                        nc.sync.dma_start(out=dstap, in_=ev[0:P2, :, :])
    nc.compile()
    return nc


def _host_prep(x, y, w_img, b_img, w_fea, b_fea, w1, w2):
    f32 = np.float32
    bf16 = ml_dtypes.bfloat16
    weff = (w2.astype(np.float64) @ w1.astype(np.float64))  # (81, 81)
    wm = np.concatenate([weff.T, weff.sum(axis=1)[None, :]], axis=0)
    wm = wm.astype(f32).astype(bf16)

    def pairw(w):
        blk = np.zeros((128, 128), dtype=f32)
        blk[0:64, 0:64] = w.T
        blk[64:128, 64:128] = w.T
        return blk.astype(bf16)

    wi = pairw(w_img.astype(f32))
    wf = pairw(w_fea.astype(f32))
    # bias row in (sc, lp)-major order: value b[sc % 64] repeated NLP times
    bi = np.tile(np.concatenate([b_img, b_img]).astype(f32), NLP)[None, :]
    bf_ = np.tile(np.concatenate([b_fea, b_fea]).astype(f32), NLP)[None, :]
    bi = bi.astype(bf16)
    bf_ = bf_.astype(bf16)

    def unf_pairs(t):  # (1, 64, 72, 108, 108) -> per-core (128, 648, 81)
        u = np.ascontiguousarray(
            t.reshape(C, NCORES, P, LLOC, P).transpose(1, 0, 3, 2, 4)
        ).reshape(NCORES, C, LLOC, P2)
        out = []
        for k in range(NCORES):
            v = u[k].reshape(C, 2, LP, P2).transpose(1, 0, 2, 3)  # slot-halves
            out.append(np.ascontiguousarray(v.reshape(128, LP, P2)).astype(bf16))
        return out

    xps = unf_pairs(np.asarray(x, dtype=f32))
    yps = unf_pairs(np.asarray(y, dtype=f32))
    shared = {"wi": wi, "wf": wf, "wm": wm, "bi": bi, "bf": bf_}
    return [dict(shared, xp=xps[k], yp=yps[k]) for k in range(NCORES)]


def kernel(x, y, w_img, b_img, w_fea, b_fea, w1, w2):
    if "nc" not in _cache:
        _cache["nc"] = _build_nc()
    nc = _cache["nc"]
    in_maps = _host_prep(x, y, w_img, b_img, w_fea, b_fea, w1, w2)
    trace = bool(os.environ.get("KERNEL_TRACE"))
    res = run_bass_kernel_spmd(
        nc, in_maps, list(range(NCORES)), trace=trace
    )
    _cache["last_result"] = res
    out = np.empty((1, C, D, H, W), dtype=np.float32)
    ov = out.reshape(C, D, HW)
    for k in range(NCORES):
        # out_d is (81, 64, 1296) with l = 648*slot + lp (already global l)
        att = res.results[k]["out"].astype(np.float32).transpose(1, 2, 0)
        blk = att.reshape(C, LLOC, P, P).transpose(0, 2, 1, 3).reshape(C, P, HW)
        ov[:, P * k:P * (k + 1), :] = blk
    return out


# revision 21
# speedup vs baseline: 1.0934x; 1.0934x over previous
"""Trainium2 Bass kernel for nn_Cross_attention_3 (sparse_attention).

Sharding: the (D, H*W) plane is unfolded into 9x9 patches; D=72 gives 8
patch-row blocks of 9 rows — exactly one per NeuronCore.  The only
cross-core dependency is the AdaptiveAvgPool over the patch axis
(bins of 128 patches straddle core boundaries); cores exchange 8-patch
half-block partial sums via a 3.4MB AllGather.

The two MLP linears have no nonlinearity between them, so they collapse
into a single 81x81 matrix; the conv bias rides along as an 82nd
contraction row whose rhs holds b[c].  The 1x1x1 conv is computed with
the patch data as the matmul's stationary operand, so its output lands
directly in (patch-element, channel) layout — the transpose the rest of
the pipeline needs comes for free.  Patches are packed in slot-halves
(slot 0 = patches 0..647, slot 1 = 648..1295) so pooling windows and
attention output runs stay contiguous.
"""

import os
import sys

import numpy as np

try:
    import ml_dtypes
except ImportError:
    ml_dtypes = None

try:
    import concourse.bacc as _  # noqa: F401
except ImportError:  # container default path
    sys.path.insert(0, "/opt/trn_rl_repo")

import concourse.bacc as bacc
import concourse.mybir as mybir
from concourse.bass_utils import run_bass_kernel_spmd
from concourse.tile import TileContext

P = 9
P2 = 81
C = 64
D = 72
H = W = 108
HW = H * W
NCORES = 8
LLOC = HW // P   # 1296 patches per core
LP = LLOC // 2   # 648 patch pairs per core (slot halves)

NLP = 24                     # chunk size in pairs, both passes
NCH_A = LP // NLP            # 27 chunks
RING_B = 216                 # pass-B ring, in pairs
NRING = LP // RING_B         # 3
NSUB = RING_B // NLP         # 9
NHALF = LP // 8              # 81 half-blocks (8 patches) per slot

F32 = mybir.dt.float32
BF16 = mybir.dt.bfloat16

_cache = {}


def _build_nc():
    nc = bacc.Bacc(None, target_bir_lowering=False, debug=False)
    xp_d = nc.declare_dram_parameter("xp", [128, LP, P2], BF16, isOutput=False)
    yp_d = nc.declare_dram_parameter("yp", [128, LP, P2], BF16, isOutput=False)
    wi_d = nc.declare_dram_parameter("wi", [128, 128], BF16, isOutput=False)
    wf_d = nc.declare_dram_parameter("wf", [128, 128], BF16, isOutput=False)
    wm_d = nc.declare_dram_parameter("wm", [82, P2], BF16, isOutput=False)
    bi_d = nc.declare_dram_parameter("bi", [1, NLP * 128], BF16, isOutput=False)
    bf_d = nc.declare_dram_parameter("bf", [1, NLP * 128], BF16, isOutput=False)
    out_d = nc.declare_dram_parameter("out", [P2, C, 2 * LP], BF16, isOutput=True)

    # per-slot 8-patch half-block sums; gathered across cores
    gath_d = nc.dram_tensor("gath", [NCORES, P2, C, NHALF], F32,
                            addr_space="Shared")

    with nc.allow_low_precision("bf16 compute pipeline"), TileContext(nc) as tc:
        with (
            tc.tile_pool(name="const", bufs=1) as constp,
            tc.tile_pool(name="stage", bufs=3) as stagep,
            tc.tile_pool(name="mlps", bufs=2) as mlpp,
            tc.tile_pool(name="psconv", bufs=4, space="PSUM") as psconv,
            tc.tile_pool(name="psmlp", bufs=2, space="PSUM") as psmlp,
            tc.tile_pool(name="dram", bufs=1, space="DRAM") as dramp,
        ):
            wi_sb = constp.tile([128, 128], BF16, tag="wi")
            wf_sb = constp.tile([128, 128], BF16, tag="wf")
            wm_sb = constp.tile([82, P2], BF16, tag="wm")
            pooled = constp.tile([P2, C, P2], BF16, tag="pooled")
            h_dram = dramp.tile([P2, C, NHALF], F32)
            nc.sync.dma_start(out=wi_sb[:, :], in_=wi_d[:, :])
            nc.sync.dma_start(out=wf_sb[:, :], in_=wf_d[:, :])
            nc.sync.dma_start(out=wm_sb[:, :], in_=wm_d[:, :])

            def conv_mlp(src_d, w_sb, b_d, lp0, dst, dst_lp0, dst_nlp, act_ix):
                """conv+MLP+lrelu for NLP pairs starting at pair lp0 of src_d.
                dst: (81, dst_nlp, 128) bf16 tile, (lp, sc)-major, written at
                lp offset dst_lp0."""
                st = stagep.tile([128, NLP, P2], BF16, tag="stage")
                nc.sync.dma_start(out=st[:, :, :], in_=src_d[:, lp0:lp0 + NLP, :])
                ms = mlpp.tile([82, NLP, 128], BF16, tag="ms")
                nc.sync.dma_start(
                    out=ms[81:82, :, :].rearrange("p a b -> p (a b)"),
                    in_=b_d[:, :],
                )
                # conv: 4 pairs per PSUM bank, N=128 each; one contiguous
                # evict per bank into ms (lp-major); DVE 2/3, ACT 1/3
                for pb in range(NLP // 4):
                    ps = psconv.tile([P2, 512], F32, tag="psc")
                    for j in range(4):
                        nc.tensor.matmul(
                            ps[0:P2, 128 * j:128 * (j + 1)],
                            st[:, 4 * pb + j, :], w_sb[:, :],
                            start=True, stop=True,
                        )
                    d = ms[0:P2, 4 * pb:4 * pb + 4, :].rearrange("p a b -> p (a b)")
                    if pb % 2 == 0:
                        nc.vector.tensor_copy(d, ps[0:P2, 0:512])
                    else:
                        nc.scalar.copy(d, ps[0:P2, 0:512])
                # MLP (+bias row) and LeakyReLU on ACT; 512 cols = 4 lp
                flat = ms[:, :, :].rearrange("p a b -> p (a b)")
                for q in range(NLP // 4):
                    mp = psmlp.tile([P2, 512], F32, tag="psm")
                    nc.tensor.matmul(
                        mp[0:P2, :], wm_sb[:, :], flat[:, 512 * q:512 * (q + 1)],
                        start=True, stop=True,
                    )
                    dq = dst[0:P2, dst_lp0 + 4 * q:dst_lp0 + 4 * q + 4,
                             :].rearrange("p a b -> p (a b)")
                    nc.scalar.activation(
                        dq, mp[0:P2, :],
                        mybir.ActivationFunctionType.Prelu, alpha=0.2,
                    )

            # ---------------- pass A: fea (y) + pooled ----------------------
            with (
                tc.tile_pool(name="feaout", bufs=2) as feap,
                tc.tile_pool(name="pooltmp", bufs=2) as ptp,
                tc.tile_pool(name="hsb", bufs=1) as hsbp,
                tc.tile_pool(name="comb", bufs=2) as combp,
            ):
                # h_sb: 8-patch half-block sums, (81, 162, 64): dim1 =
                # slot*81 + h, dim2 = c
                h_sb = hsbp.tile([P2, 2 * NHALF, C], F32, tag="hsb")
                s_sb = hsbp.tile([P2, C, NHALF], F32, tag="ssb")
                for ch in range(NCH_A):
                    fea = feap.tile([P2, NLP, 128], BF16, tag="fea")
                    conv_mlp(yp_d, wf_sb, bf_d, ch * NLP, fea, 0, NLP, ch)
                    # pairwise tree over lp: 24 -> 12 -> 6 -> 3 half-sums
                    t1 = ptp.tile([P2, 12, 128], BF16, tag="t1")
                    f2 = fea[0:P2, :, :].rearrange("p (a two) b -> p a two b", two=2)
                    nc.vector.tensor_tensor(
                        t1[0:P2, :, :], f2[:, :, 0, :], f2[:, :, 1, :],
                        mybir.AluOpType.add,
                    )
                    t2 = ptp.tile([P2, 6, 128], BF16, tag="t2")
                    t1s = t1[0:P2, :, :].rearrange("p (a two) b -> p a two b", two=2)
                    nc.vector.tensor_tensor(
                        t2[0:P2, :, :], t1s[:, :, 0, :], t1s[:, :, 1, :],
                        mybir.AluOpType.add,
                    )
                    t3 = ptp.tile([P2, 3, 128], F32, tag="t3")
                    t2s = t2[0:P2, :, :].rearrange("p (a two) b -> p a two b", two=2)
                    nc.vector.tensor_tensor(
                        t3[0:P2, :, :], t2s[:, :, 0, :], t2s[:, :, 1, :],
                        mybir.AluOpType.add,
                    )
                    # scatter the 3 half-sums per slot into h_sb
                    for slot in range(2):
                        nc.vector.tensor_copy(
                            h_sb[0:P2, NHALF * slot + 3 * ch:
                                 NHALF * slot + 3 * ch + 3, :],
                            t3[0:P2, :, 64 * slot:64 * slot + 64],
                        )
                # merge halves into 81 16-patch blocks (S), c-major for the
                # gather: S[e, c, s]
                hv = h_sb[0:P2, :, :]
                # s in [0, 40): slot0 pairs (2s, 2s+1)
                e0 = hv[:, 0:80, :].rearrange("p (a two) c -> p a two c", two=2)
                nc.vector.tensor_tensor(
                    s_sb[0:P2, :, 0:40].rearrange("p c s -> p s c"),
                    e0[:, :, 0, :], e0[:, :, 1, :], mybir.AluOpType.add,
                )
                # s = 40: slot0 h=80 + slot1 h=0
                nc.vector.tensor_tensor(
                    s_sb[0:P2, :, 40:41].rearrange("p c s -> p s c"),
                    hv[:, 80:81, :], hv[:, 81:82, :], mybir.AluOpType.add,
                )
                # s in [41, 81): slot1 pairs (1+2t, 2+2t)
                e1 = hv[:, 82:162, :].rearrange("p (a two) c -> p a two c", two=2)
                nc.vector.tensor_tensor(
                    s_sb[0:P2, :, 41:81].rearrange("p c s -> p s c"),
                    e1[:, :, 0, :], e1[:, :, 1, :], mybir.AluOpType.add,
                )
                nc.gpsimd.dma_start(out=h_dram[:, :, :], in_=s_sb[:, :, :])
                nc.gpsimd.collective_compute(
                    "AllGather",
                    mybir.AluOpType.bypass,
                    replica_groups=[list(range(NCORES))],
                    ins=[h_dram[:, :, :]],
                    outs=[gath_d[:, :, :, :]],
                )
                # bins of 128 = 8 consecutive global 16-blocks (s_g = 81k + s)
                for cc in range(8):
                    tcb = combp.tile([P2, 8, NCORES * NHALF], F32, tag="tcb")
                    for k in range(NCORES):
                        nc.gpsimd.dma_start(
                            out=tcb[0:P2, :, NHALF * k:NHALF * (k + 1)],
                            in_=gath_d[k, :, 8 * cc:8 * (cc + 1), :],
                        )
                    pr = combp.tile([P2, 8, P2], F32, tag="pr")
                    nc.vector.tensor_reduce(
                        pr[0:P2, :, :],
                        tcb[0:P2, :, :].rearrange("p c (j m) -> p c j m", m=8),
                        mybir.AxisListType.X,
                        mybir.AluOpType.add,
                    )
                    nc.vector.tensor_scalar_mul(
                        pooled[0:P2, 8 * cc:8 * (cc + 1), :], pr[0:P2, :, :],
                        1.0 / 128.0,
                    )

            # ---------------- pass B: img (x) + attention -------------------
            with (
                tc.tile_pool(name="imgring", bufs=2) as imgp,
                tc.tile_pool(name="attev", bufs=4) as attevp,
                tc.tile_pool(name="psatt", bufs=2, space="PSUM") as psatt,
            ):
                for ring in range(NRING):
                    img = imgp.tile([P2, RING_B, 128], BF16, tag="img")
                    for sub in range(NSUB):
                        conv_mlp(
                            xp_d, wi_sb, bi_d,
                            ring * RING_B + sub * NLP, img, sub * NLP, RING_B,
                            sub,
                        )
                    lp0 = ring * RING_B
                    ncols = RING_B * 2
                    for c in range(C):
                        ap = psatt.tile([P2, 512], F32, tag="psa")
                        # rhs cols (slot, lp): l = 648*slot + lp0 + lp
                        rhs = img[0:P2, :, :].rearrange(
                            "p l (s c) -> p c s l", s=2
                        )[:, c:c + 1, :, :]
                        nc.tensor.matmul(
                            ap[0:P2, 0:ncols], pooled[:, c:c + 1, :], rhs,
                            start=True, stop=True,
                        )
                        ev = attevp.tile([P2, 2, RING_B], BF16, tag="attev")
                        src = ap[0:P2, 0:ncols].rearrange("p (s l) -> p s l", s=2)
                        if c % 2 == 0:
                            nc.vector.tensor_copy(ev[0:P2, :, :], src)
                        else:
                            nc.scalar.copy(ev[0:P2, :, :], src)
                        # out[e, c, 648*slot + lp0 : +RING_B]
                        dstap = out_d[0:P2, c:c + 1, :].rearrange(
                            "p o (s l) -> p o s l", s=2
                        )[:, :, :, lp0:lp0 + RING_B]
                        nc.sync.dma_start(out=dstap, in_=ev[0:P2, :, :])
    nc.compile()
    return nc


def _host_prep(x, y, w_img, b_img, w_fea, b_fea, w1, w2):
    f32 = np.float32
    bf16 = ml_dtypes.bfloat16
    weff = (w2.astype(np.float64) @ w1.astype(np.float64))  # (81, 81)
    wm = np.concatenate([weff.T, weff.sum(axis=1)[None, :]], axis=0)
    wm = wm.astype(f32).astype(bf16)

    def pairw(w):
        blk = np.zeros((128, 128), dtype=f32)
        blk[0:64, 0:64] = w.T
        blk[64:128, 64:128] = w.T
        return blk.astype(bf16)

    wi = pairw(w_img.astype(f32))
    wf = pairw(w_fea.astype(f32))
    # bias row in (sc, lp)-major order: value b[sc % 64] repeated NLP times
    bi = np.tile(np.concatenate([b_img, b_img]).astype(f32), NLP)[None, :]
    bf_ = np.tile(np.concatenate([b_fea, b_fea]).astype(f32), NLP)[None, :]
    bi = bi.astype(bf16)
    bf_ = bf_.astype(bf16)

    def unf_pairs(t):  # (1, 64, 72, 108, 108) -> per-core (128, 648, 81)
        u = np.ascontiguousarray(
            t.reshape(C, NCORES, P, LLOC, P).transpose(1, 0, 3, 2, 4)
        ).reshape(NCORES, C, LLOC, P2)
        out = []
        for k in range(NCORES):
            v = u[k].reshape(C, 2, LP, P2).transpose(1, 0, 2, 3)  # slot-halves
            out.append(np.ascontiguousarray(v.reshape(128, LP, P2)).astype(bf16))
        return out

    xps = unf_pairs(np.asarray(x, dtype=f32))
    yps = unf_pairs(np.asarray(y, dtype=f32))
    shared = {"wi": wi, "wf": wf, "wm": wm, "bi": bi, "bf": bf_}
    return [dict(shared, xp=xps[k], yp=yps[k]) for k in range(NCORES)]


def kernel(x, y, w_img, b_img, w_fea, b_fea, w1, w2):
    if "nc" not in _cache:
        _cache["nc"] = _build_nc()
    nc = _cache["nc"]
    in_maps = _host_prep(x, y, w_img, b_img, w_fea, b_fea, w1, w2)
    trace = bool(os.environ.get("KERNEL_TRACE"))
    res = run_bass_kernel_spmd(
        nc, in_maps, list(range(NCORES)), trace=trace
    )
    _cache["last_result"] = res
    out = np.empty((1, C, D, H, W), dtype=np.float32)
    ov = out.reshape(C, D, HW)
    for k in range(NCORES):
        # out_d is (81, 64, 1296) with l = 648*slot + lp (already global l)
        att = res.results[k]["out"].astype(np.float32).transpose(1, 2, 0)
        blk = att.reshape(C, LLOC, P, P).transpose(0, 2, 1, 3).reshape(C, P, HW)
        ov[:, P * k:P * (k + 1), :] = blk
    return out


# revision 22
# speedup vs baseline: 1.1097x; 1.0149x over previous
"""Trainium2 Bass kernel for nn_Cross_attention_3 (sparse_attention).

Sharding: the (D, H*W) plane is unfolded into 9x9 patches; D=72 gives 8
patch-row blocks of 9 rows — exactly one per NeuronCore.  The only
cross-core dependency is the AdaptiveAvgPool over the patch axis
(bins of 128 patches straddle core boundaries); cores exchange 8-patch
half-block partial sums via a 3.4MB AllGather.

The two MLP linears have no nonlinearity between them, so they collapse
into a single 81x81 matrix; the conv bias rides along as an 82nd
contraction row whose rhs holds b[c].  The 1x1x1 conv is computed with
the patch data as the matmul's stationary operand, so its output lands
directly in (patch-element, channel) layout — the transpose the rest of
the pipeline needs comes for free.  Patches are packed in slot-halves
(slot 0 = patches 0..647, slot 1 = 648..1295) so pooling windows and
attention output runs stay contiguous.
"""

import os
import sys

import numpy as np

try:
    import ml_dtypes
except ImportError:
    ml_dtypes = None

try:
    import concourse.bacc as _  # noqa: F401
except ImportError:  # container default path
    sys.path.insert(0, "/opt/trn_rl_repo")

import concourse.bacc as bacc
import concourse.mybir as mybir
from concourse.bass_utils import run_bass_kernel_spmd
from concourse.tile import TileContext

P = 9
P2 = 81
C = 64
D = 72
H = W = 108
HW = H * W
NCORES = 8
LLOC = HW // P   # 1296 patches per core
LP = LLOC // 2   # 648 patch pairs per core (slot halves)

NLP = 24                     # chunk size in pairs, both passes
NCH_A = LP // NLP            # 27 chunks
RING_B = 216                 # pass-B ring, in pairs
NRING = LP // RING_B         # 3
NSUB = RING_B // NLP         # 9
NHALF = LP // 8              # 81 half-blocks (8 patches) per slot

F32 = mybir.dt.float32
BF16 = mybir.dt.bfloat16

_cache = {}


def _build_nc():
    nc = bacc.Bacc(None, target_bir_lowering=False, debug=False)
    xp_d = nc.declare_dram_parameter("xp", [128, LP, P2], BF16, isOutput=False)
    yp_d = nc.declare_dram_parameter("yp", [128, LP, P2], BF16, isOutput=False)
    wi_d = nc.declare_dram_parameter("wi", [128, 128], BF16, isOutput=False)
    wf_d = nc.declare_dram_parameter("wf", [128, 128], BF16, isOutput=False)
    wm_d = nc.declare_dram_parameter("wm", [82, P2], BF16, isOutput=False)
    bi_d = nc.declare_dram_parameter("bi", [1, NLP * 128], BF16, isOutput=False)
    bf_d = nc.declare_dram_parameter("bf", [1, NLP * 128], BF16, isOutput=False)
    out_d = nc.declare_dram_parameter("out", [P2, C, 2 * LP], BF16, isOutput=True)

    # per-slot 8-patch half-block sums; gathered across cores
    gath_d = nc.dram_tensor("gath", [NCORES, P2, C, NHALF], F32,
                            addr_space="Shared")

    with nc.allow_low_precision("bf16 compute pipeline"), TileContext(nc) as tc:
        with (
            tc.tile_pool(name="const", bufs=1) as constp,
            tc.tile_pool(name="stage", bufs=3) as stagep,
            tc.tile_pool(name="mlps", bufs=2) as mlpp,
            tc.tile_pool(name="psconv", bufs=4, space="PSUM") as psconv,
            tc.tile_pool(name="psmlp", bufs=2, space="PSUM") as psmlp,
            tc.tile_pool(name="dram", bufs=1, space="DRAM") as dramp,
        ):
            wi_sb = constp.tile([128, 128], BF16, tag="wi")
            wf_sb = constp.tile([128, 128], BF16, tag="wf")
            wm_sb = constp.tile([82, P2], BF16, tag="wm")
            pooled = constp.tile([P2, C, P2], BF16, tag="pooled")
            h_dram = dramp.tile([P2, C, NHALF], F32)
            nc.sync.dma_start(out=wi_sb[:, :], in_=wi_d[:, :])
            nc.sync.dma_start(out=wf_sb[:, :], in_=wf_d[:, :])
            nc.sync.dma_start(out=wm_sb[:, :], in_=wm_d[:, :])

            def conv_mlp(src_d, w_sb, b_d, lp0, dst, dst_lp0, dst_nlp, act_ix):
                """conv+MLP+lrelu for NLP pairs starting at pair lp0 of src_d.
                dst: (81, dst_nlp, 128) bf16 tile, (lp, sc)-major, written at
                lp offset dst_lp0."""
                st = stagep.tile([128, NLP, P2], BF16, tag="stage")
                nc.sync.dma_start(out=st[:, :, :], in_=src_d[:, lp0:lp0 + NLP, :])
                ms = mlpp.tile([82, NLP, 128], BF16, tag="ms")
                nc.sync.dma_start(
                    out=ms[81:82, :, :].rearrange("p a b -> p (a b)"),
                    in_=b_d[:, :],
                )
                # conv: 4 pairs per PSUM bank, N=128 each; one contiguous
                # evict per bank into ms (lp-major); DVE 2/3, ACT 1/3
                for pb in range(NLP // 4):
                    ps = psconv.tile([P2, 512], F32, tag="psc")
                    for j in range(4):
                        nc.tensor.matmul(
                            ps[0:P2, 128 * j:128 * (j + 1)],
                            st[:, 4 * pb + j, :], w_sb[:, :],
                            start=True, stop=True,
                        )
                    d = ms[0:P2, 4 * pb:4 * pb + 4, :].rearrange("p a b -> p (a b)")
                    if pb % 2 == 0:
                        nc.vector.tensor_copy(d, ps[0:P2, 0:512])
                    else:
                        nc.scalar.copy(d, ps[0:P2, 0:512])
                # MLP (+bias row) and LeakyReLU on ACT; 512 cols = 4 lp
                flat = ms[:, :, :].rearrange("p a b -> p (a b)")
                for q in range(NLP // 4):
                    mp = psmlp.tile([P2, 512], F32, tag="psm")
                    nc.tensor.matmul(
                        mp[0:P2, :], wm_sb[:, :], flat[:, 512 * q:512 * (q + 1)],
                        start=True, stop=True,
                    )
                    dq = dst[0:P2, dst_lp0 + 4 * q:dst_lp0 + 4 * q + 4,
                             :].rearrange("p a b -> p (a b)")
                    nc.scalar.activation(
                        dq, mp[0:P2, :],
                        mybir.ActivationFunctionType.Prelu, alpha=0.2,
                    )

            # ---------------- pass A: fea (y) + pooled ----------------------
            with (
                tc.tile_pool(name="feaout", bufs=2) as feap,
                tc.tile_pool(name="pooltmp", bufs=2) as ptp,
                tc.tile_pool(name="hsb", bufs=1) as hsbp,
                tc.tile_pool(name="comb", bufs=2) as combp,
            ):
                # h_sb: 8-patch half-block sums, (81, 162, 64): dim1 =
                # slot*81 + h, dim2 = c
                h_sb = hsbp.tile([P2, 2 * NHALF, C], F32, tag="hsb")
                s_sb = hsbp.tile([P2, C, NHALF], F32, tag="ssb")
                for ch in range(NCH_A):
                    fea = feap.tile([P2, NLP, 128], BF16, tag="fea")
                    conv_mlp(yp_d, wf_sb, bf_d, ch * NLP, fea, 0, NLP, ch)
                    # pairwise tree over lp: 24 -> 12 -> 6 -> 3 half-sums
                    t1 = ptp.tile([P2, 12, 128], BF16, tag="t1")
                    f2 = fea[0:P2, :, :].rearrange("p (a two) b -> p a two b", two=2)
                    nc.vector.tensor_tensor(
                        t1[0:P2, :, :], f2[:, :, 0, :], f2[:, :, 1, :],
                        mybir.AluOpType.add,
                    )
                    t2 = ptp.tile([P2, 6, 128], BF16, tag="t2")
                    t1s = t1[0:P2, :, :].rearrange("p (a two) b -> p a two b", two=2)
                    nc.vector.tensor_tensor(
                        t2[0:P2, :, :], t1s[:, :, 0, :], t1s[:, :, 1, :],
                        mybir.AluOpType.add,
                    )
                    t3 = ptp.tile([P2, 3, 128], F32, tag="t3")
                    t2s = t2[0:P2, :, :].rearrange("p (a two) b -> p a two b", two=2)
                    nc.vector.tensor_tensor(
                        t3[0:P2, :, :], t2s[:, :, 0, :], t2s[:, :, 1, :],
                        mybir.AluOpType.add,
                    )
                    # scatter the 3 half-sums per slot into h_sb
                    for slot in range(2):
                        nc.vector.tensor_copy(
                            h_sb[0:P2, NHALF * slot + 3 * ch:
                                 NHALF * slot + 3 * ch + 3, :],
                            t3[0:P2, :, 64 * slot:64 * slot + 64],
                        )
                # merge halves into 81 16-patch blocks (S), c-major for the
                # gather: S[e, c, s]
                hv = h_sb[0:P2, :, :]
                # s in [0, 40): slot0 pairs (2s, 2s+1)
                e0 = hv[:, 0:80, :].rearrange("p (a two) c -> p a two c", two=2)
                nc.vector.tensor_tensor(
                    s_sb[0:P2, :, 0:40].rearrange("p c s -> p s c"),
                    e0[:, :, 0, :], e0[:, :, 1, :], mybir.AluOpType.add,
                )
                # s = 40: slot0 h=80 + slot1 h=0
                nc.vector.tensor_tensor(
                    s_sb[0:P2, :, 40:41].rearrange("p c s -> p s c"),
                    hv[:, 80:81, :], hv[:, 81:82, :], mybir.AluOpType.add,
                )
                # s in [41, 81): slot1 pairs (1+2t, 2+2t)
                e1 = hv[:, 82:162, :].rearrange("p (a two) c -> p a two c", two=2)
                nc.vector.tensor_tensor(
                    s_sb[0:P2, :, 41:81].rearrange("p c s -> p s c"),
                    e1[:, :, 0, :], e1[:, :, 1, :], mybir.AluOpType.add,
                )
                nc.gpsimd.dma_start(out=h_dram[:, :, :], in_=s_sb[:, :, :])
                nc.gpsimd.collective_compute(
                    "AllGather",
                    mybir.AluOpType.bypass,
                    replica_groups=[list(range(NCORES))],
                    ins=[h_dram[:, :, :]],
                    outs=[gath_d[:, :, :, :]],
                )
                # bins of 128 = 8 consecutive global 16-blocks (s_g = 81k + s)
                for cc in range(8):
                    tcb = combp.tile([P2, 8, NCORES * NHALF], F32, tag="tcb")
                    for k in range(NCORES):
                        nc.gpsimd.dma_start(
                            out=tcb[0:P2, :, NHALF * k:NHALF * (k + 1)],
                            in_=gath_d[k, :, 8 * cc:8 * (cc + 1), :],
                        )
                    pr = combp.tile([P2, 8, P2], F32, tag="pr")
                    nc.vector.tensor_reduce(
                        pr[0:P2, :, :],
                        tcb[0:P2, :, :].rearrange("p c (j m) -> p c j m", m=8),
                        mybir.AxisListType.X,
                        mybir.AluOpType.add,
                    )
                    nc.vector.tensor_scalar_mul(
                        pooled[0:P2, 8 * cc:8 * (cc + 1), :], pr[0:P2, :, :],
                        1.0 / 128.0,
                    )

            # ---------------- pass B: img (x) + attention -------------------
            with (
                tc.tile_pool(name="imgring", bufs=1) as imgp,
                tc.tile_pool(name="attev", bufs=4) as attevp,
                tc.tile_pool(name="psatt", bufs=2, space="PSUM") as psatt,
            ):
                for ring in range(NRING):
                    img = imgp.tile([P2, RING_B, 128], BF16, tag="img")
                    for sub in range(NSUB):
                        conv_mlp(
                            xp_d, wi_sb, bi_d,
                            ring * RING_B + sub * NLP, img, sub * NLP, RING_B,
                            sub,
                        )
                    lp0 = ring * RING_B
                    ncols = RING_B * 2
                    for c in range(C):
                        ap = psatt.tile([P2, 512], F32, tag="psa")
                        # rhs cols (slot, lp): l = 648*slot + lp0 + lp
                        rhs = img[0:P2, :, :].rearrange(
                            "p l (s c) -> p c s l", s=2
                        )[:, c:c + 1, :, :]
                        nc.tensor.matmul(
                            ap[0:P2, 0:ncols], pooled[:, c:c + 1, :], rhs,
                            start=True, stop=True,
                        )
                        ev = attevp.tile([P2, 2, RING_B], BF16, tag="attev")
                        src = ap[0:P2, 0:ncols].rearrange("p (s l) -> p s l", s=2)
                        if c % 2 == 0:
                            nc.vector.tensor_copy(ev[0:P2, :, :], src)
                        else:
                            nc.scalar.copy(ev[0:P2, :, :], src)
                        # out[e, c, 648*slot + lp0 : +RING_B]
                        dstap = out_d[0:P2, c:c + 1, :].rearrange(
                            "p o (s l) -> p o s l", s=2
                        )[:, :, :, lp0:lp0 + RING_B]
                        nc.sync.dma_start(out=dstap, in_=ev[0:P2, :, :])
    nc.compile()
    return nc


def _host_prep(x, y, w_img, b_img, w_fea, b_fea, w1, w2):
    f32 = np.float32
    bf16 = ml_dtypes.bfloat16
    weff = (w2.astype(np.float64) @ w1.astype(np.float64))  # (81, 81)
    wm = np.concatenate([weff.T, weff.sum(axis=1)[None, :]], axis=0)
    wm = wm.astype(f32).astype(bf16)

    def pairw(w):
        blk = np.zeros((128, 128), dtype=f32)
        blk[0:64, 0:64] = w.T
        blk[64:128, 64:128] = w.T
        return blk.astype(bf16)

    wi = pairw(w_img.astype(f32))
    wf = pairw(w_fea.astype(f32))
    # bias row in (sc, lp)-major order: value b[sc % 64] repeated NLP times
    bi = np.tile(np.concatenate([b_img, b_img]).astype(f32), NLP)[None, :]
    bf_ = np.tile(np.concatenate([b_fea, b_fea]).astype(f32), NLP)[None, :]
    bi = bi.astype(bf16)
    bf_ = bf_.astype(bf16)

    def unf_pairs(t):  # (1, 64, 72, 108, 108) -> per-core (128, 648, 81)
        u = np.ascontiguousarray(
            t.reshape(C, NCORES, P, LLOC, P).transpose(1, 0, 3, 2, 4)
        ).reshape(NCORES, C, LLOC, P2)
        out = []
        for k in range(NCORES):
            v = u[k].reshape(C, 2, LP, P2).transpose(1, 0, 2, 3)  # slot-halves
            out.append(np.ascontiguousarray(v.reshape(128, LP, P2)).astype(bf16))
        return out

    xps = unf_pairs(np.asarray(x, dtype=f32))
    yps = unf_pairs(np.asarray(y, dtype=f32))
    shared = {"wi": wi, "wf": wf, "wm": wm, "bi": bi, "bf": bf_}
    return [dict(shared, xp=xps[k], yp=yps[k]) for k in range(NCORES)]


def kernel(x, y, w_img, b_img, w_fea, b_fea, w1, w2):
    if "nc" not in _cache:
        _cache["nc"] = _build_nc()
    nc = _cache["nc"]
    in_maps = _host_prep(x, y, w_img, b_img, w_fea, b_fea, w1, w2)
    trace = bool(os.environ.get("KERNEL_TRACE"))
    res = run_bass_kernel_spmd(
        nc, in_maps, list(range(NCORES)), trace=trace
    )
    _cache["last_result"] = res
    out = np.empty((1, C, D, H, W), dtype=np.float32)
    ov = out.reshape(C, D, HW)
    for k in range(NCORES):
        # out_d is (81, 64, 1296) with l = 648*slot + lp (already global l)
        att = res.results[k]["out"].astype(np.float32).transpose(1, 2, 0)
        blk = att.reshape(C, LLOC, P, P).transpose(0, 2, 1, 3).reshape(C, P, HW)
        ov[:, P * k:P * (k + 1), :] = blk
    return out
